# revision 9
# baseline (speedup 1.0000x reference)
"""Trainium2 Bass kernel for DiagonalSSMLayer.

Math: y = C_w @ h + D*u  where  h[l] = lam*h[l-1] + (B_w @ u)[l]  (per state
channel, lam = sigmoid(log_lambda)).  The reference computes the causal
exponential-decay convolution via FFT; here it is the exact linear recurrence,
done with the DVE's native tensor_tensor_scan.

Sharding: 8 cores = (batch b in 0..3) x (sequence half s in 0..1).
Each core gets u[b, s*2048:(s+1)*2048, :] transposed to [D=1024, 2048] so the
contraction dim d sits on SBUF partitions for both GEMMs (out = lhsT.T @ rhs
contracts over the partition dim).  GEMMs run in float32r (full PE rate).

Cross-half carry: second-half cores prepend a HALO of the last `HALO`
positions of the first half and run the scan through it, which reconstructs
the incoming state up to a factor lam^HALO <= 3.4e-5 -- far below the
float32r rounding noise (~2e-4).  First-half cores get a zero halo, making
the program uniform across cores (SPMD).  Optional exact mode ("ar") moves
the true carry with a pairwise AllReduce instead; it is ~25us slower due to
collective latency at the end of the critical path.

Output is computed transposed (yT = [1024, 2048]) per core, fusing
y = C@h + D*u via scalar_tensor_tensor with D as per-partition scalars, and
un-transposed on the host.
"""

import numpy as np

B, L, DM, NS = 4, 4096, 1024, 256
HALF = L // 2          # 2048 sequence positions per core
NCORES = 8
LC = 512               # l-chunk (matmul free dim / scan chunk)
NLC = HALF // LC       # 4 main chunks
HALO = 1024
NHC = HALO // LC       # 2 halo chunks
KT = DM // 128         # 8 k-tiles (contraction over d)
NT = NS // 128         # 2 n-tiles (state channels)

_CACHE = {}


def _build_halo():
    from concourse import bacc, tile, mybir

    MULT = mybir.AluOpType.mult
    ADD = mybir.AluOpType.add
    f32 = mybir.dt.float32
    f32r = mybir.dt.float32r

    nc = bacc.Bacc("TRN2", target_bir_lowering=False, debug=False,
                   num_devices=NCORES)

    # uT carries [halo | main] columns: [DM, HALO + HALF]
    uT_d = nc.dram_tensor("uT", [DM, HALO + HALF], f32r, kind="ExternalInput").ap()
    BwT_d = nc.dram_tensor("BwT", [DM, NS], f32r, kind="ExternalInput").ap()
    CwT_d = nc.dram_tensor("CwT", [NS, DM], f32r, kind="ExternalInput").ap()
    lam_d = nc.dram_tensor("lamblk", [NS, LC], f32, kind="ExternalInput").ap()
    dvec_d = nc.dram_tensor("dvec", [DM, 1], f32, kind="ExternalInput").ap()
    yT_d = nc.dram_tensor("yT", [DM, HALF], f32, kind="ExternalOutput").ap()

    with tile.TileContext(nc) as tc:
        with tc.tile_pool(name="const", bufs=1) as cpool, \
             tc.tile_pool(name="u", bufs=1) as upool, \
             tc.tile_pool(name="uh", bufs=8) as uhpool, \
             tc.tile_pool(name="h", bufs=1) as hpool, \
             tc.tile_pool(name="y", bufs=6) as ypool, \
             tc.tile_pool(name="bu_ps", bufs=4, space="PSUM") as bupool, \
             tc.tile_pool(name="y_ps", bufs=4, space="PSUM") as yppool:

            # ---- front constants: only what GEMM1/scan need
            BwT_sb = [cpool.tile([128, NS], f32r, name=f"bw{k}") for k in range(KT)]
            for k in range(KT):
                nc.gpsimd.dma_start(out=BwT_sb[k][:], in_=BwT_d[k * 128:(k + 1) * 128, :])
            lam_sb = [cpool.tile([128, LC], f32, name=f"lam{n}") for n in range(NT)]
            for n in range(NT):
                nc.gpsimd.dma_start(out=lam_sb[n][:], in_=lam_d[n * 128:(n + 1) * 128, :])

            hr = [hpool.tile([128, HALF], f32r, name=f"hr_{n}") for n in range(NT)]
            hh = [[hpool.tile([128, LC], f32r, name=f"hh{n}_{c}") for c in range(NHC)]
                  for n in range(NT)]

            # ---- halo: GEMM1 + scan over the carry-reconstruction region
            for j in range(NHC):
                uh = []
                for k in range(KT):
                    t = uhpool.tile([128, LC], f32r, tag="uh")
                    nc.sync.dma_start(out=t[:], in_=uT_d[k * 128:(k + 1) * 128,
                                                         j * LC:(j + 1) * LC])
                    uh.append(t)
                for n in range(NT):
                    bu_ps = bupool.tile([128, LC], f32, tag="bu")
                    for k in range(KT):
                        nc.tensor.matmul(bu_ps[:],
                                         BwT_sb[k][:, n * 128:(n + 1) * 128],
                                         uh[k][:],
                                         start=(k == 0), stop=(k == KT - 1))
                    init = 0.0 if j == 0 else hh[n][j - 1][:, LC - 1:LC]
                    nc.vector.tensor_tensor_scan(
                        hh[n][j][:], lam_sb[n][:], bu_ps[:], init, MULT, ADD)

            # ---- deferred constants (needed by GEMM2 only)
            CwT_sb = [cpool.tile([128, DM], f32r, name=f"cw{n}") for n in range(NT)]
            for n in range(NT):
                nc.gpsimd.dma_start(out=CwT_sb[n][:], in_=CwT_d[n * 128:(n + 1) * 128, :])
            dvec_sb = [cpool.tile([128, 1], f32, name=f"dv{k}") for k in range(KT)]
            for k in range(KT):
                nc.gpsimd.dma_start(out=dvec_sb[k][:], in_=dvec_d[k * 128:(k + 1) * 128, :])

            # ---- main chunks, with GEMM2/y-mat software-pipelined one chunk
            # behind the scan chain so the next scan never queues behind the
            # previous chunk's y-materialize ops on the (in-order) DVE.
            uT_sb = [[upool.tile([128, LC], f32r, name=f"u{k}_{j}")
                      for j in range(NLC)] for k in range(KT)]

            def gemm2(j):
                for k in range(KT):
                    y_ps = yppool.tile([128, LC], f32, tag="y")
                    for n in range(NT):
                        nc.tensor.matmul(y_ps[:],
                                         CwT_sb[n][:, k * 128:(k + 1) * 128],
                                         hr[n][:, j * LC:(j + 1) * LC],
                                         start=(n == 0), stop=(n == NT - 1))
                    y_sb = ypool.tile([128, LC], f32, tag="ysb")
                    nc.vector.scalar_tensor_tensor(
                        y_sb[:], uT_sb[k][j][:].bitcast(f32),
                        dvec_sb[k][:], y_ps[:], MULT, ADD)
                    nc.scalar.dma_start(out=yT_d[k * 128:(k + 1) * 128,
                                                 j * LC:(j + 1) * LC],
                                        in_=y_sb[:])

            for j in range(NLC):
                for k in range(KT):
                    nc.sync.dma_start(out=uT_sb[k][j][:],
                                      in_=uT_d[k * 128:(k + 1) * 128,
                                               HALO + j * LC:HALO + (j + 1) * LC])
                for n in range(NT):
                    bu_ps = bupool.tile([128, LC], f32, tag="bu")
                    for k in range(KT):
                        nc.tensor.matmul(bu_ps[:],
                                         BwT_sb[k][:, n * 128:(n + 1) * 128],
                                         uT_sb[k][j][:],
                                         start=(k == 0), stop=(k == KT - 1))
                    init = (hh[n][NHC - 1][:, LC - 1:LC] if j == 0
                            else hr[n][:, j * LC - 1:j * LC])
                    nc.vector.tensor_tensor_scan(
                        hr[n][:, j * LC:(j + 1) * LC],
                        lam_sb[n][:], bu_ps[:], init, MULT, ADD)
                if j > 0:
                    gemm2(j - 1)
            gemm2(NLC - 1)

    nc.compile()
    return nc


def _sigmoid(x):
    return 1.0 / (1.0 + np.exp(-x))


def kernel(u, log_lambda, B_w, C_w, D):
    from concourse.bass_utils import run_bass_kernel_spmd

    if "nc" not in _CACHE:
        _CACHE["nc"] = _build_halo()
    nc = _CACHE["nc"]

    u = np.asarray(u, dtype=np.float32)
    lam = _sigmoid(np.asarray(log_lambda, dtype=np.float64))
    BwT = np.ascontiguousarray(np.asarray(B_w, np.float32).T)      # [D, N]
    CwT = np.ascontiguousarray(np.asarray(C_w, np.float32).T)      # [N, D]
    dvec = np.ascontiguousarray(np.asarray(D, np.float32).reshape(DM, 1))
    lamblk = np.ascontiguousarray(
        np.broadcast_to(lam[:, None], (NS, LC))).astype(np.float32)

    in_maps = []
    for core in range(NCORES):
        b, s = core // 2, core % 2
        uTh = np.zeros((DM, HALO + HALF), dtype=np.float32)
        if s == 1:
            uTh[:, :HALO] = u[b, HALF - HALO:HALF, :].T
        uTh[:, HALO:] = u[b, s * HALF:(s + 1) * HALF, :].T
        in_maps.append({
            "uT": uTh,
            "BwT": BwT,
            "CwT": CwT,
            "lamblk": lamblk,
            "dvec": dvec,
        })
    _CACHE["in_maps"] = in_maps

    def _run():
        return run_bass_kernel_spmd(nc, in_maps, core_ids=list(range(NCORES)))

    try:
        res = _run()
    except Exception:
        # a previously failed execution can wedge the backend; reset + retry
        try:
            import ctypes, jax
            jax.devices()
            lib = ctypes.CDLL("/opt/axon/libaxon_pjrt.so")
            lib.axon_reset.restype = ctypes.c_int64
            lib.axon_reset()
        except Exception:
            pass
        res = _run()

    y = np.empty((B, L, DM), dtype=np.float32)
    for core in range(NCORES):
        b, s = core // 2, core % 2
        y[b, s * HALF:(s + 1) * HALF, :] = res.results[core]["yT"].T
    return y


# revision 11
# speedup vs baseline: 1.1124x; 1.1124x over previous
"""Trainium2 Bass kernel for DiagonalSSMLayer.

Math: y = C_w @ h + D*u  where  h[l] = lam*h[l-1] + (B_w @ u)[l]  (per state
channel, lam = sigmoid(log_lambda)).  The reference computes the causal
exponential-decay convolution via FFT; here it is the exact linear recurrence,
done with the DVE's native tensor_tensor_scan.

Sharding: 8 cores = (batch b in 0..3) x (sequence half s in 0..1).
Each core gets u[b, s*2048:(s+1)*2048, :] transposed to [D=1024, 2048] so the
contraction dim d sits on SBUF partitions for both GEMMs (out = lhsT.T @ rhs
contracts over the partition dim).  GEMMs run in float32r (full PE rate).

Cross-half carry: second-half cores prepend a HALO of the last `HALO`
positions of the first half and run the scan through it, which reconstructs
the incoming state up to a factor lam^HALO <= 3.4e-5 -- far below the
float32r rounding noise (~2e-4).  First-half cores get a zero halo, making
the program uniform across cores (SPMD).  Optional exact mode ("ar") moves
the true carry with a pairwise AllReduce instead; it is ~25us slower due to
collective latency at the end of the critical path.

Output is computed transposed (yT = [1024, 2048]) per core, fusing
y = C@h + D*u via scalar_tensor_tensor with D as per-partition scalars, and
un-transposed on the host.
"""

import numpy as np

B, L, DM, NS = 4, 4096, 1024, 256
HALF = L // 2          # 2048 sequence positions per core
NCORES = 8
LC = 512               # l-chunk (matmul free dim / scan chunk)
NLC = HALF // LC       # 4 main chunks
HALO = 1024
NHC = HALO // LC       # 2 halo chunks
KT = DM // 128         # 8 k-tiles (contraction over d)
NT = NS // 128         # 2 n-tiles (state channels)

_CACHE = {}


def _build_halo():
    from concourse import bacc, tile, mybir

    MULT = mybir.AluOpType.mult
    ADD = mybir.AluOpType.add
    f32 = mybir.dt.float32
    f32r = mybir.dt.float32r

    nc = bacc.Bacc("TRN2", target_bir_lowering=False, debug=False,
                   num_devices=NCORES)

    # uT carries [halo | main] columns: [DM, HALO + HALF]
    uT_d = nc.dram_tensor("uT", [DM, HALO + HALF], f32r, kind="ExternalInput").ap()
    BwT_d = nc.dram_tensor("BwT", [DM, NS], f32r, kind="ExternalInput").ap()
    CwT_d = nc.dram_tensor("CwT", [NS, DM], f32r, kind="ExternalInput").ap()
    lam_d = nc.dram_tensor("lamvec", [NS, 1], f32, kind="ExternalInput").ap()
    dvec_d = nc.dram_tensor("dvec", [DM, 1], f32, kind="ExternalInput").ap()
    yT_d = nc.dram_tensor("yT", [DM, HALF], f32, kind="ExternalOutput").ap()

    with tile.TileContext(nc) as tc:
        with tc.tile_pool(name="const", bufs=1) as cpool, \
             tc.tile_pool(name="u", bufs=1) as upool, \
             tc.tile_pool(name="uh", bufs=1) as uhpool, \
             tc.tile_pool(name="h", bufs=1) as hpool, \
             tc.tile_pool(name="y", bufs=6) as ypool, \
             tc.tile_pool(name="bu_ps", bufs=3, space="PSUM") as bupool, \
             tc.tile_pool(name="y_ps", bufs=5, space="PSUM") as yppool:

            # ---- front constants: only what GEMM1/scan need
            BwT_sb = [cpool.tile([128, NS], f32r, name=f"bw{k}") for k in range(KT)]
            for k in range(KT):
                nc.gpsimd.dma_start(out=BwT_sb[k][:], in_=BwT_d[k * 128:(k + 1) * 128, :])
            lam_sb = [cpool.tile([128, LC], f32, name=f"lam{n}") for n in range(NT)]
            lamv_sb = [cpool.tile([128, 1], f32, name=f"lamv{n}") for n in range(NT)]
            for n in range(NT):
                nc.gpsimd.dma_start(out=lamv_sb[n][:], in_=lam_d[n * 128:(n + 1) * 128, :])
                nc.vector.memset(lam_sb[n][:], 1.0)
                nc.vector.tensor_scalar_mul(lam_sb[n][:], lam_sb[n][:], lamv_sb[n][:])

            hr = [hpool.tile([128, HALF], f32r, name=f"hr_{n}") for n in range(NT)]
            hh = [[hpool.tile([128, LC], f32r, name=f"hh{n}_{c}") for c in range(NHC)]
                  for n in range(NT)]

            # ---- halo: GEMM1 + scan over the carry-reconstruction region
            uh2 = []
            for k in range(KT):
                t = uhpool.tile([128, HALO], f32r, name=f"uh{k}")
                nc.sync.dma_start(out=t[:], in_=uT_d[k * 128:(k + 1) * 128, 0:HALO])
                uh2.append(t)
            for j in range(NHC):
                uh = [t[:, j * LC:(j + 1) * LC] for t in uh2]
                for n in range(NT):
                    bu_ps = bupool.tile([128, LC], f32, tag="bu")
                    for k in range(KT):
                        nc.tensor.matmul(bu_ps[:],
                                         BwT_sb[k][:, n * 128:(n + 1) * 128],
                                         uh[k],
                                         start=(k == 0), stop=(k == KT - 1))
                    init = 0.0 if j == 0 else hh[n][j - 1][:, LC - 1:LC]
                    nc.vector.tensor_tensor_scan(
                        hh[n][j][:], lam_sb[n][:], bu_ps[:], init, MULT, ADD)

            # ---- deferred constants (needed by GEMM2 only)
            CwT_sb = [cpool.tile([128, DM], f32r, name=f"cw{n}") for n in range(NT)]
            for n in range(NT):
                nc.gpsimd.dma_start(out=CwT_sb[n][:], in_=CwT_d[n * 128:(n + 1) * 128, :])
            dvec_sb = [cpool.tile([128, 1], f32, name=f"dv{k}") for k in range(KT)]
            for k in range(KT):
                nc.gpsimd.dma_start(out=dvec_sb[k][:], in_=dvec_d[k * 128:(k + 1) * 128, :])

            # ---- main chunks, with GEMM2/y-mat software-pipelined one chunk
            # behind the scan chain so the next scan never queues behind the
            # previous chunk's y-materialize ops on the (in-order) DVE.
            uT_sb = [[upool.tile([128, LC], f32r, name=f"u{k}_{j}")
                      for j in range(NLC)] for k in range(KT)]

            def gemm2(j):
                for k in range(KT):
                    y_ps = yppool.tile([128, LC], f32, tag="y")
                    for n in range(NT):
                        nc.tensor.matmul(y_ps[:],
                                         CwT_sb[n][:, k * 128:(k + 1) * 128],
                                         hr[n][:, j * LC:(j + 1) * LC],
                                         start=(n == 0), stop=(n == NT - 1))
                    y_sb = ypool.tile([128, LC], f32, tag="ysb")
                    nc.vector.scalar_tensor_tensor(
                        y_sb[:], uT_sb[k][j][:].bitcast(f32),
                        dvec_sb[k][:], y_ps[:], MULT, ADD)
                    nc.scalar.dma_start(out=yT_d[k * 128:(k + 1) * 128,
                                                 j * LC:(j + 1) * LC],
                                        in_=y_sb[:])

            for j in range(NLC):
                for k in range(KT):
                    nc.sync.dma_start(out=uT_sb[k][j][:],
                                      in_=uT_d[k * 128:(k + 1) * 128,
                                               HALO + j * LC:HALO + (j + 1) * LC])
                for n in range(NT):
                    bu_ps = bupool.tile([128, LC], f32, tag="bu")
                    for k in range(KT):
                        nc.tensor.matmul(bu_ps[:],
                                         BwT_sb[k][:, n * 128:(n + 1) * 128],
                                         uT_sb[k][j][:],
                                         start=(k == 0), stop=(k == KT - 1))
                    init = (hh[n][NHC - 1][:, LC - 1:LC] if j == 0
                            else hr[n][:, j * LC - 1:j * LC])
                    nc.vector.tensor_tensor_scan(
                        hr[n][:, j * LC:(j + 1) * LC],
                        lam_sb[n][:], bu_ps[:], init, MULT, ADD)
                if j > 0:
                    gemm2(j - 1)
            gemm2(NLC - 1)

    nc.compile()
    return nc


def _sigmoid(x):
    return 1.0 / (1.0 + np.exp(-x))


def kernel(u, log_lambda, B_w, C_w, D):
    from concourse.bass_utils import run_bass_kernel_spmd

    if "nc" not in _CACHE:
        _CACHE["nc"] = _build_halo()
    nc = _CACHE["nc"]

    u = np.asarray(u, dtype=np.float32)
    lam = _sigmoid(np.asarray(log_lambda, dtype=np.float64))
    BwT = np.ascontiguousarray(np.asarray(B_w, np.float32).T)      # [D, N]
    CwT = np.ascontiguousarray(np.asarray(C_w, np.float32).T)      # [N, D]
    dvec = np.ascontiguousarray(np.asarray(D, np.float32).reshape(DM, 1))
    lamvec = np.ascontiguousarray(lam.reshape(NS, 1)).astype(np.float32)

    in_maps = []
    for core in range(NCORES):
        b, s = core // 2, core % 2
        uTh = np.zeros((DM, HALO + HALF), dtype=np.float32)
        if s == 1:
            uTh[:, :HALO] = u[b, HALF - HALO:HALF, :].T
        uTh[:, HALO:] = u[b, s * HALF:(s + 1) * HALF, :].T
        in_maps.append({
            "uT": uTh,
            "BwT": BwT,
            "CwT": CwT,
            "lamvec": lamvec,
            "dvec": dvec,
        })
    _CACHE["in_maps"] = in_maps

    def _run():
        return run_bass_kernel_spmd(nc, in_maps, core_ids=list(range(NCORES)))

    try:
        res = _run()
    except Exception:
        # a previously failed execution can wedge the backend; reset + retry
        try:
            import ctypes, jax
            jax.devices()
            lib = ctypes.CDLL("/opt/axon/libaxon_pjrt.so")
            lib.axon_reset.restype = ctypes.c_int64
            lib.axon_reset()
        except Exception:
            pass
        res = _run()

    y = np.empty((B, L, DM), dtype=np.float32)
    for core in range(NCORES):
        b, s = core // 2, core % 2
        y[b, s * HALF:(s + 1) * HALF, :] = res.results[core]["yT"].T
    return y


# revision 14
# speedup vs baseline: 1.1714x; 1.0531x over previous
"""Trainium2 Bass kernel for DiagonalSSMLayer.

Math: y = C_w @ h + D*u  where  h[l] = lam*h[l-1] + (B_w @ u)[l]  (per state
channel, lam = sigmoid(log_lambda)).  The reference computes the causal
exponential-decay convolution via FFT; here it is the exact linear recurrence,
done with the DVE's native tensor_tensor_scan.

Sharding: 8 cores = (batch b in 0..3) x (sequence half s in 0..1).
Each core gets u[b, s*2048:(s+1)*2048, :] transposed to [D=1024, 2048] so the
contraction dim d sits on SBUF partitions for both GEMMs (out = lhsT.T @ rhs
contracts over the partition dim).  GEMMs run in float32r (full PE rate).

Cross-half carry: second-half cores prepend a HALO of the last `HALO`
positions of the first half and run the scan through it, which reconstructs
the incoming state up to a factor lam^HALO <= 3.4e-5 -- far below the
float32r rounding noise (~2e-4).  First-half cores get a zero halo, making
the program uniform across cores (SPMD).  Optional exact mode ("ar") moves
the true carry with a pairwise AllReduce instead; it is ~25us slower due to
collective latency at the end of the critical path.

Output is computed transposed (yT = [1024, 2048]) per core, fusing
y = C@h + D*u via scalar_tensor_tensor with D as per-partition scalars, and
un-transposed on the host.
"""

import numpy as np

B, L, DM, NS = 4, 4096, 1024, 256
HALF = L // 2          # 2048 sequence positions per core
NCORES = 8
LC = 512               # l-chunk (matmul free dim / scan chunk)
NLC = HALF // LC       # 4 main chunks
HALO = 1024
NHC = HALO // LC       # 2 halo chunks
KT = DM // 128         # 8 k-tiles (contraction over d)
NT = NS // 128         # 2 n-tiles (state channels)

_CACHE = {}


def _build_halo():
    from concourse import bacc, tile, mybir

    MULT = mybir.AluOpType.mult
    ADD = mybir.AluOpType.add
    f32 = mybir.dt.float32
    f32r = mybir.dt.float32r

    nc = bacc.Bacc("TRN2", target_bir_lowering=False, debug=False,
                   num_devices=NCORES)

    # uT carries [halo | main] columns: [DM, HALO + HALF]
    uT_d = nc.dram_tensor("uT", [DM, HALO + HALF], f32r, kind="ExternalInput").ap()
    BwT_d = nc.dram_tensor("BwT", [DM, NS], f32r, kind="ExternalInput").ap()
    CwT_d = nc.dram_tensor("CwT", [NS, DM], f32r, kind="ExternalInput").ap()
    lam_d = nc.dram_tensor("lamvec", [NS, 1], f32, kind="ExternalInput").ap()
    dvec_d = nc.dram_tensor("dvec", [DM, 1], f32, kind="ExternalInput").ap()
    yT_d = nc.dram_tensor("yT", [DM, HALF], f32, kind="ExternalOutput").ap()

    with tile.TileContext(nc) as tc:
        with tc.tile_pool(name="const", bufs=1) as cpool, \
             tc.tile_pool(name="u", bufs=1) as upool, \
             tc.tile_pool(name="uh", bufs=1) as uhpool, \
             tc.tile_pool(name="h", bufs=1) as hpool, \
             tc.tile_pool(name="y", bufs=6) as ypool, \
             tc.tile_pool(name="bu_ps", bufs=3, space="PSUM") as bupool, \
             tc.tile_pool(name="y_ps", bufs=5, space="PSUM") as yppool:

            # ---- PE warmup: ~10us of dummy matmuls at t=0 trips the HAM
            # clock-gate to 8/8 (2.4 GHz); later inter-matmul gaps stay under
            # the ~3.4us MID window so the PE never re-throttles.
            warm_sb = cpool.tile([128, 512], f32r, name="warm")
            nc.gpsimd.memset(warm_sb[:].bitcast(f32), 1.0)
            warm_ps = yppool.tile([128, LC], f32, tag="y")
            for w in range(24):
                nc.tensor.matmul(warm_ps[:], warm_sb[:, 0:128], warm_sb[:],
                                 start=(w == 0), stop=(w == 23))

            # ---- front constants: only what GEMM1/scan need
            BwT_sb = [cpool.tile([128, NS], f32r, name=f"bw{k}") for k in range(KT)]
            for k in range(KT):
                nc.gpsimd.dma_start(out=BwT_sb[k][:], in_=BwT_d[k * 128:(k + 1) * 128, :])
            lam_sb = [cpool.tile([128, LC], f32, name=f"lam{n}") for n in range(NT)]
            lamv_sb = [cpool.tile([128, 1], f32, name=f"lamv{n}") for n in range(NT)]
            for n in range(NT):
                nc.gpsimd.dma_start(out=lamv_sb[n][:], in_=lam_d[n * 128:(n + 1) * 128, :])
                nc.vector.memset(lam_sb[n][:], 1.0)
                nc.vector.tensor_scalar_mul(lam_sb[n][:], lam_sb[n][:], lamv_sb[n][:])

            hr = [hpool.tile([128, HALF], f32r, name=f"hr_{n}") for n in range(NT)]
            hh = [[hpool.tile([128, LC], f32r, name=f"hh{n}_{c}") for c in range(NHC)]
                  for n in range(NT)]

            # ---- halo: GEMM1 + scan over the carry-reconstruction region
            uh2 = []
            for k in range(KT):
                t = uhpool.tile([128, HALO], f32r, name=f"uh{k}")
                nc.sync.dma_start(out=t[:], in_=uT_d[k * 128:(k + 1) * 128, 0:HALO])
                uh2.append(t)
            for j in range(NHC):
                uh = [t[:, j * LC:(j + 1) * LC] for t in uh2]
                for n in range(NT):
                    bu_ps = bupool.tile([128, LC], f32, tag="bu")
                    for k in range(KT):
                        nc.tensor.matmul(bu_ps[:],
                                         BwT_sb[k][:, n * 128:(n + 1) * 128],
                                         uh[k],
                                         start=(k == 0), stop=(k == KT - 1))
                    init = 0.0 if j == 0 else hh[n][j - 1][:, LC - 1:LC]
                    nc.vector.tensor_tensor_scan(
                        hh[n][j][:], lam_sb[n][:], bu_ps[:], init, MULT, ADD)

            # ---- deferred constants (needed by GEMM2 only)
            CwT_sb = [cpool.tile([128, DM], f32r, name=f"cw{n}") for n in range(NT)]
            for n in range(NT):
                nc.gpsimd.dma_start(out=CwT_sb[n][:], in_=CwT_d[n * 128:(n + 1) * 128, :])
            dvec_sb = [cpool.tile([128, 1], f32, name=f"dv{k}") for k in range(KT)]
            for k in range(KT):
                nc.gpsimd.dma_start(out=dvec_sb[k][:], in_=dvec_d[k * 128:(k + 1) * 128, :])

            # ---- main chunks, with GEMM2/y-mat software-pipelined one chunk
            # behind the scan chain so the next scan never queues behind the
            # previous chunk's y-materialize ops on the (in-order) DVE.
            uT2_sb = [[upool.tile([128, 2 * LC], f32r, name=f"u{k}_{p}")
                       for p in range(NLC // 2)] for k in range(KT)]
            uT_sb = [[uT2_sb[k][j // 2][:, (j % 2) * LC:(j % 2 + 1) * LC]
                      for j in range(NLC)] for k in range(KT)]

            def gemm2(j):
                for k in range(KT):
                    y_ps = yppool.tile([128, LC], f32, tag="y")
                    for n in range(NT):
                        nc.tensor.matmul(y_ps[:],
                                         CwT_sb[n][:, k * 128:(k + 1) * 128],
                                         hr[n][:, j * LC:(j + 1) * LC],
                                         start=(n == 0), stop=(n == NT - 1))
                    y_sb = ypool.tile([128, LC], f32, tag="ysb")
                    nc.vector.scalar_tensor_tensor(
                        y_sb[:], uT_sb[k][j].bitcast(f32),
                        dvec_sb[k][:], y_ps[:], MULT, ADD)
                    nc.scalar.dma_start(out=yT_d[k * 128:(k + 1) * 128,
                                                 j * LC:(j + 1) * LC],
                                        in_=y_sb[:])

            for j in range(NLC):
                if j % 2 == 0:
                    for k in range(KT):
                        nc.sync.dma_start(
                            out=uT2_sb[k][j // 2][:],
                            in_=uT_d[k * 128:(k + 1) * 128,
                                     HALO + j * LC:HALO + (j + 2) * LC])
                for n in range(NT):
                    bu_ps = bupool.tile([128, LC], f32, tag="bu")
                    for k in range(KT):
                        nc.tensor.matmul(bu_ps[:],
                                         BwT_sb[k][:, n * 128:(n + 1) * 128],
                                         uT_sb[k][j],
                                         start=(k == 0), stop=(k == KT - 1))
                    init = (hh[n][NHC - 1][:, LC - 1:LC] if j == 0
                            else hr[n][:, j * LC - 1:j * LC])
                    nc.vector.tensor_tensor_scan(
                        hr[n][:, j * LC:(j + 1) * LC],
                        lam_sb[n][:], bu_ps[:], init, MULT, ADD)
                if j > 0:
                    gemm2(j - 1)
            gemm2(NLC - 1)

    nc.compile()
    return nc


def _sigmoid(x):
    return 1.0 / (1.0 + np.exp(-x))


def kernel(u, log_lambda, B_w, C_w, D):
    from concourse.bass_utils import run_bass_kernel_spmd

    if "nc" not in _CACHE:
        _CACHE["nc"] = _build_halo()
    nc = _CACHE["nc"]

    u = np.asarray(u, dtype=np.float32)
    lam = _sigmoid(np.asarray(log_lambda, dtype=np.float64))
    BwT = np.ascontiguousarray(np.asarray(B_w, np.float32).T)      # [D, N]
    CwT = np.ascontiguousarray(np.asarray(C_w, np.float32).T)      # [N, D]
    dvec = np.ascontiguousarray(np.asarray(D, np.float32).reshape(DM, 1))
    lamvec = np.ascontiguousarray(lam.reshape(NS, 1)).astype(np.float32)

    in_maps = []
    for core in range(NCORES):
        b, s = core // 2, core % 2
        uTh = np.zeros((DM, HALO + HALF), dtype=np.float32)
        if s == 1:
            uTh[:, :HALO] = u[b, HALF - HALO:HALF, :].T
        uTh[:, HALO:] = u[b, s * HALF:(s + 1) * HALF, :].T
        in_maps.append({
            "uT": uTh,
            "BwT": BwT,
            "CwT": CwT,
            "lamvec": lamvec,
            "dvec": dvec,
        })
    _CACHE["in_maps"] = in_maps

    def _run():
        return run_bass_kernel_spmd(nc, in_maps, core_ids=list(range(NCORES)))

    try:
        res = _run()
    except Exception:
        # a previously failed execution can wedge the backend; reset + retry
        try:
            import ctypes, jax
            jax.devices()
            lib = ctypes.CDLL("/opt/axon/libaxon_pjrt.so")
            lib.axon_reset.restype = ctypes.c_int64
            lib.axon_reset()
        except Exception:
            pass
        res = _run()

    y = np.empty((B, L, DM), dtype=np.float32)
    for core in range(NCORES):
        b, s = core // 2, core % 2
        y[b, s * HALF:(s + 1) * HALF, :] = res.results[core]["yT"].T
    return y
